# revision 5
# baseline (speedup 1.0000x reference)
"""MoE feed-forward (B=4,S=2048,D=1024,F=2048,E=8,top-2) on 8 trn2 NeuronCores.

Strategy (expert-parallel, per sharding hint):
 - Host computes the top-2 softmax routing (tiny: [T,1024]@[1024,8]) and
   dispatches tokens: core e receives the tokens routed to expert e,
   transposed to [D, C] (C = max token count over experts, zero padded).
 - The combine weight w_t is folded into the up-projection input
   (y = (silu(x@W1) * ((w*x)@W3)) @ W2 = w * expert(x)), so the device
   kernel is a pure grouped SwiGLU GEMM chain and the host combine is an
   unweighted scatter-add.
 - Device (per core): h1 = W1^T x, h3 = W3^T (w x), g = silu(h1)*h3,
   y^T = W2^T g, streamed over column blocks of <=512 tokens.
   All matmuls fp16 x fp16 -> fp32 PSUM.
"""

import numpy as np

import concourse.tile as tile
from concourse import bacc, mybir
from concourse.bass_utils import run_bass_kernel_spmd

B, S, D, F, E, TOPK = 4, 2048, 1024, 2048, 8, 2
N_CORES = 8
KD = D // 128   # 8 contraction tiles for D
KF = F // 128   # 16 contraction tiles for F

_nc_cache = {}


def _build_nc(C):
    """Build the per-core Bass program for token capacity C."""
    f16 = mybir.dt.float16
    f32 = mybir.dt.float32
    blocks = [512] * (C // 512) + ([C % 512] if C % 512 else [])

    nc = bacc.Bacc(None, target_bir_lowering=False)
    xT = nc.dram_tensor("xT", [D, C], f16, kind="ExternalInput")
    xwT = nc.dram_tensor("xwT", [D, C], f16, kind="ExternalInput")
    W1 = nc.dram_tensor("W1", [D, F], f16, kind="ExternalInput")
    W3 = nc.dram_tensor("W3", [D, F], f16, kind="ExternalInput")
    W2 = nc.dram_tensor("W2", [F, D], f16, kind="ExternalInput")
    yT = nc.dram_tensor("yT", [D, C], f32, kind="ExternalOutput")

    with tile.TileContext(nc) as tc:
        with (
            tc.tile_pool(name="wpool", bufs=1) as wpool,
            tc.tile_pool(name="xpool", bufs=2) as xpool,
            tc.tile_pool(name="gpool", bufs=2) as gpool,
            tc.tile_pool(name="spool", bufs=3) as spool,
            tc.tile_pool(name="ypool", bufs=3) as ypool,
            tc.tile_pool(name="psA", bufs=2, space="PSUM") as psA,
            tc.tile_pool(name="psB", bufs=2, space="PSUM") as psB,
            tc.tile_pool(name="psY", bufs=2, space="PSUM") as psY,
        ):
            # Resident weights: W1/W3 as [128, F] row tiles (partition = D
            # contraction dim), W2 as [128, D] row tiles (partition = F).
            w1sb, w3sb, w2sb = [], [], []
            for k in range(KD):
                t = wpool.tile([128, F], f16, tag=f"w1_{k}")
                nc.sync.dma_start(out=t, in_=W1[k * 128:(k + 1) * 128, :])
                w1sb.append(t)
            for k in range(KD):
                t = wpool.tile([128, F], f16, tag=f"w3_{k}")
                nc.sync.dma_start(out=t, in_=W3[k * 128:(k + 1) * 128, :])
                w3sb.append(t)
            for k in range(KF):
                t = wpool.tile([128, D], f16, tag=f"w2_{k}")
                nc.sync.dma_start(out=t, in_=W2[k * 128:(k + 1) * 128, :])
                w2sb.append(t)

            c0 = 0
            for nb in blocks:
                xsb, xwsb = [], []
                for k in range(KD):
                    t = xpool.tile([128, 512], f16, tag=f"x{k}")
                    nc.sync.dma_start(
                        out=t[:, :nb], in_=xT[k * 128:(k + 1) * 128, c0:c0 + nb]
                    )
                    xsb.append(t)
                    t2 = xpool.tile([128, 512], f16, tag=f"xw{k}")
                    nc.sync.dma_start(
                        out=t2[:, :nb], in_=xwT[k * 128:(k + 1) * 128, c0:c0 + nb]
                    )
                    xwsb.append(t2)

                gts = []
                for f in range(KF):
                    fs = slice(f * 128, (f + 1) * 128)
                    ps1 = psA.tile([128, 512], f32, tag="ps1")
                    for k in range(KD):
                        nc.tensor.matmul(
                            ps1[:, :nb], lhsT=w1sb[k][:, fs], rhs=xsb[k][:, :nb],
                            start=(k == 0), stop=(k == KD - 1),
                        )
                    ps3 = psB.tile([128, 512], f32, tag="ps3")
                    for k in range(KD):
                        nc.tensor.matmul(
                            ps3[:, :nb], lhsT=w3sb[k][:, fs], rhs=xwsb[k][:, :nb],
                            start=(k == 0), stop=(k == KD - 1),
                        )
                    s = spool.tile([128, 512], f16, tag="s")
                    nc.scalar.activation(
                        s[:, :nb], ps1[:, :nb], mybir.ActivationFunctionType.Silu
                    )
                    g = gpool.tile([128, 512], f16, tag=f"g{f}")
                    nc.vector.tensor_mul(g[:, :nb], s[:, :nb], ps3[:, :nb])
                    gts.append(g)

                for dd in range(KD):
                    ds_ = slice(dd * 128, (dd + 1) * 128)
                    psy = psY.tile([128, 512], f32, tag="psy")
                    for f in range(KF):
                        nc.tensor.matmul(
                            psy[:, :nb], lhsT=w2sb[f][:, ds_], rhs=gts[f][:, :nb],
                            start=(f == 0), stop=(f == KF - 1),
                        )
                    yt = ypool.tile([128, 512], f32, tag="y")
                    nc.scalar.copy(yt[:, :nb], psy[:, :nb])
                    nc.sync.dma_start(
                        out=yT[dd * 128:(dd + 1) * 128, c0:c0 + nb], in_=yt[:, :nb]
                    )
                c0 += nb
    nc.finalize()
    return nc


def _route(x, Wg):
    """Top-2 softmax routing in float64 (top-2/top-3 gaps are >>f32 eps, so
    this matches the f32 reference selection exactly)."""
    logits = x.astype(np.float64) @ Wg.astype(np.float64)
    logits -= logits.max(axis=-1, keepdims=True)
    g = np.exp(logits)
    g /= g.sum(axis=-1, keepdims=True)
    top_i = np.argpartition(-g, TOPK - 1, axis=-1)[:, :TOPK]      # [T, 2]
    tg = np.take_along_axis(g, top_i, axis=-1)
    tg = tg / tg.sum(axis=-1, keepdims=True)
    return top_i, tg


def run(inputs, trace=False, trace_cores=None):
    hidden_states = np.asarray(inputs["hidden_states"], dtype=np.float32)
    Wg = np.asarray(inputs["Wg"], dtype=np.float32)
    W1 = np.asarray(inputs["W1"], dtype=np.float32)
    W3 = np.asarray(inputs["W3"], dtype=np.float32)
    W2 = np.asarray(inputs["W2"], dtype=np.float32)

    x = hidden_states.reshape(-1, D)                              # [T, D]
    T = x.shape[0]
    top_i, tg = _route(x, Wg)

    idx = []
    wts = []
    for e in range(E):
        sel = top_i == e                                          # [T, 2]
        rows = np.where(sel.any(axis=-1))[0]
        idx.append(rows)
        wts.append(np.where(sel[rows, 0], tg[rows, 0], tg[rows, 1]))
    counts = [len(r) for r in idx]
    C = max(max(counts), 1)

    if C not in _nc_cache:
        _nc_cache[C] = _build_nc(C)
    nc = _nc_cache[C]

    in_maps = []
    for e in range(E):
        rows = idx[e]
        c = len(rows)
        xg = x[rows]                                              # [c, D] f32
        xTe = np.zeros((D, C), np.float16)
        xTe[:, :c] = xg.T
        xwTe = np.zeros((D, C), np.float16)
        xwTe[:, :c] = (xg * wts[e][:, None].astype(np.float32)).T
        in_maps.append({
            "xT": xTe,
            "xwT": xwTe,
            "W1": W1[e].astype(np.float16),
            "W3": W3[e].astype(np.float16),
            "W2": W2[e].astype(np.float16),
        })

    kwargs = {}
    if trace:
        kwargs["trace"] = True
        kwargs["trace_cores"] = trace_cores or list(range(N_CORES))
    res = run_bass_kernel_spmd(nc, in_maps, list(range(N_CORES)), **kwargs)

    out = np.zeros((T, D), np.float32)
    for e in range(E):
        c = len(idx[e])
        if c:
            out[idx[e]] += res.results[e]["yT"][:, :c].T
    return out.reshape(B, S, D), res


def kernel(**inputs):
    out, _ = run(inputs, trace=False)
    return out


# revision 6
# speedup vs baseline: 1.1210x; 1.1210x over previous
"""MoE feed-forward (B=4,S=2048,D=1024,F=2048,E=8,top-2) on 8 trn2 NeuronCores.

Strategy (expert-parallel, per sharding hint):
 - Host computes the top-2 softmax routing (tiny: [T,1024]@[1024,8]) and
   dispatches tokens: core e receives the tokens routed to expert e,
   transposed to [D, C] (C = max token count over experts, zero padded).
 - The combine weight w_t is folded into the up-projection input
   (y = (silu(x@W1) * ((w*x)@W3)) @ W2 = w * expert(x)), so the device
   kernel is a pure grouped SwiGLU GEMM chain and the host combine is an
   unweighted scatter-add.
 - Device (per core): h1 = W1^T x, h3 = W3^T (w x), g = silu(h1)*h3,
   y^T = W2^T g, streamed over column blocks of <=512 tokens.
   All matmuls fp16 x fp16 -> fp32 PSUM.
"""

import numpy as np

import concourse.tile as tile
from concourse import bacc, mybir
from concourse.bass_utils import run_bass_kernel_spmd

B, S, D, F, E, TOPK = 4, 2048, 1024, 2048, 8, 2
N_CORES = 8
KD = D // 128   # 8 contraction tiles for D
KF = F // 128   # 16 contraction tiles for F

_nc_cache = {}


def _build_nc(C):
    """Build the per-core Bass program for token capacity C."""
    f16 = mybir.dt.float16
    f32 = mybir.dt.float32
    blocks = [512] * (C // 512) + ([C % 512] if C % 512 else [])

    nc = bacc.Bacc(None, target_bir_lowering=False)
    xT = nc.dram_tensor("xT", [D, C], f16, kind="ExternalInput")
    xwT = nc.dram_tensor("xwT", [D, C], f16, kind="ExternalInput")
    W1 = nc.dram_tensor("W1", [D, F], f16, kind="ExternalInput")
    W3 = nc.dram_tensor("W3", [D, F], f16, kind="ExternalInput")
    W2 = nc.dram_tensor("W2", [F, D], f16, kind="ExternalInput")
    yT = nc.dram_tensor("yT", [D, C], f32, kind="ExternalOutput")

    with tile.TileContext(nc) as tc:
        with (
            tc.tile_pool(name="wpool", bufs=1) as wpool,
            tc.tile_pool(name="xpool", bufs=2) as xpool,
            tc.tile_pool(name="gpool", bufs=2) as gpool,
            tc.tile_pool(name="spool", bufs=1) as spool,
            tc.tile_pool(name="ypool", bufs=3) as ypool,
            tc.tile_pool(name="psA", bufs=3, space="PSUM") as psA,
            tc.tile_pool(name="psB", bufs=3, space="PSUM") as psB,
            tc.tile_pool(name="psY", bufs=2, space="PSUM") as psY,
        ):
            # All data DMAs share one HW queue in program (emission) order, so
            # emission order = arrival order. Emit block-0 x first, then W1
            # (the h1 pass needs only those), then xw0/W3, then W2.
            x0 = []
            nb0 = blocks[0]
            for k in range(KD):
                t = xpool.tile([128, 512], f16, tag=f"x{k}")
                nc.sync.dma_start(out=t[:, :nb0], in_=xT[k * 128:(k + 1) * 128, :nb0])
                x0.append(t)

            # Resident weights: W1/W3 as [128, F] row tiles (partition = D
            # contraction dim), W2 as [128, D] row tiles (partition = F).
            w1sb, w3sb, w2sb = [], [], []
            for k in range(KD):
                t = wpool.tile([128, F], f16, tag=f"w1_{k}")
                nc.sync.dma_start(out=t, in_=W1[k * 128:(k + 1) * 128, :])
                w1sb.append(t)

            xw0 = []
            for k in range(KD):
                t2 = xpool.tile([128, 512], f16, tag=f"xw{k}")
                nc.sync.dma_start(out=t2[:, :nb0], in_=xwT[k * 128:(k + 1) * 128, :nb0])
                xw0.append(t2)
            for k in range(KD):
                t = wpool.tile([128, F], f16, tag=f"w3_{k}")
                nc.sync.dma_start(out=t, in_=W3[k * 128:(k + 1) * 128, :])
                w3sb.append(t)
            for k in range(KF):
                t = wpool.tile([128, D], f16, tag=f"w2_{k}")
                nc.sync.dma_start(out=t, in_=W2[k * 128:(k + 1) * 128, :])
                w2sb.append(t)

            c0 = 0
            for b, nb in enumerate(blocks):
                if b == 0:
                    xsb, xwsb = x0, xw0
                else:
                    xsb, xwsb = [], []
                    for k in range(KD):
                        t = xpool.tile([128, 512], f16, tag=f"x{k}")
                        nc.sync.dma_start(
                            out=t[:, :nb], in_=xT[k * 128:(k + 1) * 128, c0:c0 + nb]
                        )
                        xsb.append(t)
                        t2 = xpool.tile([128, 512], f16, tag=f"xw{k}")
                        nc.sync.dma_start(
                            out=t2[:, :nb], in_=xwT[k * 128:(k + 1) * 128, c0:c0 + nb]
                        )
                        xwsb.append(t2)

                # Pass 1: h1 = W1^T x, s = silu(h1)  (needs only W1 + x)
                sts = []
                for f in range(KF):
                    fs = slice(f * 128, (f + 1) * 128)
                    ps1 = psA.tile([128, 512], f32, tag="ps1")
                    for k in range(KD):
                        nc.tensor.matmul(
                            ps1[:, :nb], lhsT=w1sb[k][:, fs], rhs=xsb[k][:, :nb],
                            start=(k == 0), stop=(k == KD - 1),
                        )
                    s = spool.tile([128, 512], f16, tag=f"s{f}")
                    nc.scalar.activation(
                        s[:, :nb], ps1[:, :nb], mybir.ActivationFunctionType.Silu
                    )
                    sts.append(s)

                # Pass 2: h3 = W3^T (w*x), g = s * h3
                gts = []
                for f in range(KF):
                    fs = slice(f * 128, (f + 1) * 128)
                    ps3 = psB.tile([128, 512], f32, tag="ps3")
                    for k in range(KD):
                        nc.tensor.matmul(
                            ps3[:, :nb], lhsT=w3sb[k][:, fs], rhs=xwsb[k][:, :nb],
                            start=(k == 0), stop=(k == KD - 1),
                        )
                    g = gpool.tile([128, 512], f16, tag=f"g{f}")
                    nc.vector.tensor_mul(g[:, :nb], sts[f][:, :nb], ps3[:, :nb])
                    gts.append(g)

                # Pass 3: y^T = W2^T g
                for dd in range(KD):
                    ds_ = slice(dd * 128, (dd + 1) * 128)
                    psy = psY.tile([128, 512], f32, tag="psy")
                    for f in range(KF):
                        nc.tensor.matmul(
                            psy[:, :nb], lhsT=w2sb[f][:, ds_], rhs=gts[f][:, :nb],
                            start=(f == 0), stop=(f == KF - 1),
                        )
                    yt = ypool.tile([128, 512], f32, tag="y")
                    nc.scalar.copy(yt[:, :nb], psy[:, :nb])
                    nc.sync.dma_start(
                        out=yT[dd * 128:(dd + 1) * 128, c0:c0 + nb], in_=yt[:, :nb]
                    )
                c0 += nb
    nc.finalize()
    return nc


def _route(x, Wg):
    """Top-2 softmax routing in float64 (top-2/top-3 gaps are >>f32 eps, so
    this matches the f32 reference selection exactly)."""
    logits = x.astype(np.float64) @ Wg.astype(np.float64)
    logits -= logits.max(axis=-1, keepdims=True)
    g = np.exp(logits)
    g /= g.sum(axis=-1, keepdims=True)
    top_i = np.argpartition(-g, TOPK - 1, axis=-1)[:, :TOPK]      # [T, 2]
    tg = np.take_along_axis(g, top_i, axis=-1)
    tg = tg / tg.sum(axis=-1, keepdims=True)
    return top_i, tg


def run(inputs, trace=False, trace_cores=None):
    hidden_states = np.asarray(inputs["hidden_states"], dtype=np.float32)
    Wg = np.asarray(inputs["Wg"], dtype=np.float32)
    W1 = np.asarray(inputs["W1"], dtype=np.float32)
    W3 = np.asarray(inputs["W3"], dtype=np.float32)
    W2 = np.asarray(inputs["W2"], dtype=np.float32)

    x = hidden_states.reshape(-1, D)                              # [T, D]
    T = x.shape[0]
    top_i, tg = _route(x, Wg)

    idx = []
    wts = []
    for e in range(E):
        sel = top_i == e                                          # [T, 2]
        rows = np.where(sel.any(axis=-1))[0]
        idx.append(rows)
        wts.append(np.where(sel[rows, 0], tg[rows, 0], tg[rows, 1]))
    counts = [len(r) for r in idx]
    C = max(max(counts), 1)

    if C not in _nc_cache:
        _nc_cache[C] = _build_nc(C)
    nc = _nc_cache[C]

    in_maps = []
    for e in range(E):
        rows = idx[e]
        c = len(rows)
        xg = x[rows]                                              # [c, D] f32
        xTe = np.zeros((D, C), np.float16)
        xTe[:, :c] = xg.T
        xwTe = np.zeros((D, C), np.float16)
        xwTe[:, :c] = (xg * wts[e][:, None].astype(np.float32)).T
        in_maps.append({
            "xT": xTe,
            "xwT": xwTe,
            "W1": W1[e].astype(np.float16),
            "W3": W3[e].astype(np.float16),
            "W2": W2[e].astype(np.float16),
        })

    kwargs = {}
    if trace:
        kwargs["trace"] = True
        kwargs["trace_cores"] = trace_cores or list(range(N_CORES))
    res = run_bass_kernel_spmd(nc, in_maps, list(range(N_CORES)), **kwargs)

    out = np.zeros((T, D), np.float32)
    for e in range(E):
        c = len(idx[e])
        if c:
            out[idx[e]] += res.results[e]["yT"][:, :c].T
    return out.reshape(B, S, D), res


def kernel(**inputs):
    out, _ = run(inputs, trace=False)
    return out


# revision 8
# speedup vs baseline: 1.1305x; 1.0085x over previous
"""MoE feed-forward (B=4,S=2048,D=1024,F=2048,E=8,top-2) on 8 trn2 NeuronCores.

Strategy (expert-parallel, per sharding hint):
 - Host computes the top-2 softmax routing (tiny: [T,1024]@[1024,8]) and
   dispatches tokens: core e receives the tokens routed to expert e,
   transposed to [D, C] (C = max token count over experts, zero padded).
 - Device (per core): h1 = W1^T x, s = silu(h1), h3 = W3^T x, g = s*h3,
   y^T = (W2^T g) * w  (w = per-token combine weight, broadcast across
   partitions), streamed over column blocks of <=512 tokens.
   All matmuls fp16 x fp16 -> fp32 PSUM; y emitted fp16, combined on host
   in fp32 via unweighted scatter-add.
"""

import numpy as np

import concourse.bass as bass
import concourse.tile as tile
from concourse import bacc, mybir
from concourse.bass_utils import run_bass_kernel_spmd

B, S, D, F, E, TOPK = 4, 2048, 1024, 2048, 8, 2
N_CORES = 8
KD = D // 128   # 8 contraction tiles for D
KF = F // 128   # 16 contraction tiles for F

_nc_cache = {}


def _build_nc(C):
    """Build the per-core Bass program for token capacity C."""
    f16 = mybir.dt.float16
    f32 = mybir.dt.float32
    blocks = [512] * (C // 512) + ([C % 512] if C % 512 else [])

    nc = bacc.Bacc(None, target_bir_lowering=False, enable_partition_id=False)
    xT = nc.dram_tensor("xT", [D, C], f16, kind="ExternalInput")
    wt = nc.dram_tensor("wt", [1, C], f32, kind="ExternalInput")
    W1 = nc.dram_tensor("W1", [D, F], f16, kind="ExternalInput")
    W3 = nc.dram_tensor("W3", [D, F], f16, kind="ExternalInput")
    W2 = nc.dram_tensor("W2", [F, D], f16, kind="ExternalInput")
    yT = nc.dram_tensor("yT", [D, C], f16, kind="ExternalOutput")

    # [D, nb] slab of xT/yT viewed as [128, KD, nb] (partition-major tiles)
    def slab(t, c0, nb):
        return t[:, c0:c0 + nb].rearrange("(k p) n -> p k n", p=128)

    with tile.TileContext(nc) as tc:
        with (
            tc.tile_pool(name="wpool", bufs=1) as wpool,
            tc.tile_pool(name="xpool", bufs=2) as xpool,
            tc.tile_pool(name="gpool", bufs=2) as gpool,
            tc.tile_pool(name="spool", bufs=1) as spool,
            tc.tile_pool(name="ypool", bufs=2) as ypool,
            tc.tile_pool(name="wbpool", bufs=2) as wbpool,
            tc.tile_pool(name="psA", bufs=3, space="PSUM") as psA,
            tc.tile_pool(name="psB", bufs=3, space="PSUM") as psB,
            tc.tile_pool(name="psY", bufs=2, space="PSUM") as psY,
        ):
            # All data DMAs share one HW queue in emission order. The h1 pass
            # needs only W1 + x block 0, so emit those first (interleaved so
            # the first f-group's k-tiles land earliest), then W3/W2.
            nb0 = blocks[0]
            w1sb = []
            for k in range(KD):
                t = wpool.tile([128, F], f16, tag=f"w1_{k}")
                w1sb.append(t)
            x0 = xpool.tile([128, KD, 512], f16, tag="x")
            nc.sync.dma_start(out=w1sb[0], in_=W1[0:128, :])
            nc.sync.dma_start(out=x0[:, 0:4, :nb0], in_=slab(xT, 0, nb0)[:, 0:4, :])
            for k in range(1, 4):
                nc.sync.dma_start(out=w1sb[k], in_=W1[k * 128:(k + 1) * 128, :])
            nc.sync.dma_start(out=x0[:, 4:KD, :nb0], in_=slab(xT, 0, nb0)[:, 4:KD, :])
            for k in range(4, KD):
                nc.sync.dma_start(out=w1sb[k], in_=W1[k * 128:(k + 1) * 128, :])

            wb0 = wbpool.tile([128, 512], f32, tag="wb")
            nc.sync.dma_start(
                out=wb0[:, :nb0],
                in_=bass.AP(tensor=wt.ap().tensor, offset=0,
                            ap=[[0, 128], [1, nb0]]),
            )

            w3sb = wpool.tile([128, KD, F], f16, tag="w3")
            nc.sync.dma_start(out=w3sb, in_=W3[:, :].rearrange("(k p) n -> p k n", p=128))
            w2sb = wpool.tile([128, KF, D], f16, tag="w2")
            nc.sync.dma_start(out=w2sb, in_=W2[:, :].rearrange("(k p) n -> p k n", p=128))

            c0 = 0
            for b, nb in enumerate(blocks):
                if b == 0:
                    xsb, wb = x0, wb0
                else:
                    xsb = xpool.tile([128, KD, 512], f16, tag="x")
                    nc.sync.dma_start(out=xsb[:, :, :nb], in_=slab(xT, c0, nb))
                    wb = wbpool.tile([128, 512], f32, tag="wb")
                    nc.sync.dma_start(
                        out=wb[:, :nb],
                        in_=bass.AP(tensor=wt.ap().tensor, offset=c0,
                                    ap=[[0, 128], [1, nb]]),
                    )

                # Pass 1: h1 = W1^T x, s = silu(h1)  (needs only W1 + x)
                sts = []
                for f in range(KF):
                    fs = slice(f * 128, (f + 1) * 128)
                    ps1 = psA.tile([128, 512], f32, tag="ps1")
                    for k in range(KD):
                        nc.tensor.matmul(
                            ps1[:, :nb], lhsT=w1sb[k][:, fs], rhs=xsb[:, k, :nb],
                            start=(k == 0), stop=(k == KD - 1),
                        )
                    s = spool.tile([128, 512], f16, tag=f"s{f}")
                    nc.scalar.activation(
                        s[:, :nb], ps1[:, :nb], mybir.ActivationFunctionType.Silu
                    )
                    sts.append(s)

                # Pass 2: h3 = W3^T x, g = s * h3
                gts = []
                for f in range(KF):
                    fs = slice(f * 128, (f + 1) * 128)
                    ps3 = psB.tile([128, 512], f32, tag="ps3")
                    for k in range(KD):
                        nc.tensor.matmul(
                            ps3[:, :nb], lhsT=w3sb[:, k, fs], rhs=xsb[:, k, :nb],
                            start=(k == 0), stop=(k == KD - 1),
                        )
                    g = gpool.tile([128, 512], f16, tag=f"g{f}")
                    nc.vector.tensor_mul(g[:, :nb], sts[f][:, :nb], ps3[:, :nb])
                    gts.append(g)

                # Pass 3: y^T = (W2^T g) * w
                ysb = ypool.tile([128, KD, 512], f16, tag="y")
                for dd in range(KD):
                    ds_ = slice(dd * 128, (dd + 1) * 128)
                    psy = psY.tile([128, 512], f32, tag="psy")
                    for f in range(KF):
                        nc.tensor.matmul(
                            psy[:, :nb], lhsT=w2sb[:, f, ds_], rhs=gts[f][:, :nb],
                            start=(f == 0), stop=(f == KF - 1),
                        )
                    nc.vector.tensor_mul(ysb[:, dd, :nb], psy[:, :nb], wb[:, :nb])
                nc.sync.dma_start(out=slab(yT, c0, nb), in_=ysb[:, :, :nb])
                c0 += nb
    nc.finalize()
    return nc


def _route(x, Wg):
    """Top-2 softmax routing in float64 (top-2/top-3 gaps are >>f32 eps, so
    this matches the f32 reference selection exactly)."""
    logits = x.astype(np.float64) @ Wg.astype(np.float64)
    logits -= logits.max(axis=-1, keepdims=True)
    g = np.exp(logits)
    g /= g.sum(axis=-1, keepdims=True)
    top_i = np.argpartition(-g, TOPK - 1, axis=-1)[:, :TOPK]      # [T, 2]
    tg = np.take_along_axis(g, top_i, axis=-1)
    tg = tg / tg.sum(axis=-1, keepdims=True)
    return top_i, tg


def run(inputs, trace=False, trace_cores=None):
    hidden_states = np.asarray(inputs["hidden_states"], dtype=np.float32)
    Wg = np.asarray(inputs["Wg"], dtype=np.float32)
    W1 = np.asarray(inputs["W1"], dtype=np.float32)
    W3 = np.asarray(inputs["W3"], dtype=np.float32)
    W2 = np.asarray(inputs["W2"], dtype=np.float32)

    x = hidden_states.reshape(-1, D)                              # [T, D]
    T = x.shape[0]
    top_i, tg = _route(x, Wg)

    idx = []
    wts = []
    for e in range(E):
        sel = top_i == e                                          # [T, 2]
        rows = np.where(sel.any(axis=-1))[0]
        idx.append(rows)
        wts.append(np.where(sel[rows, 0], tg[rows, 0], tg[rows, 1]))
    counts = [len(r) for r in idx]
    C = max(max(counts), 1)

    if C not in _nc_cache:
        _nc_cache[C] = _build_nc(C)
    nc = _nc_cache[C]

    in_maps = []
    for e in range(E):
        rows = idx[e]
        c = len(rows)
        xg = x[rows]                                              # [c, D] f32
        xTe = np.zeros((D, C), np.float16)
        xTe[:, :c] = xg.T
        wte = np.zeros((1, C), np.float32)
        wte[0, :c] = wts[e]
        in_maps.append({
            "xT": xTe,
            "wt": wte,
            "W1": W1[e].astype(np.float16),
            "W3": W3[e].astype(np.float16),
            "W2": W2[e].astype(np.float16),
        })

    kwargs = {}
    if trace:
        kwargs["trace"] = True
        kwargs["trace_cores"] = trace_cores or list(range(N_CORES))
    res = run_bass_kernel_spmd(nc, in_maps, list(range(N_CORES)), **kwargs)

    out = np.zeros((T, D), np.float32)
    for e in range(E):
        c = len(idx[e])
        if c:
            out[idx[e]] += res.results[e]["yT"][:, :c].T.astype(np.float32)
    return out.reshape(B, S, D), res


def kernel(**inputs):
    out, _ = run(inputs, trace=False)
    return out


# revision 12
# speedup vs baseline: 1.1389x; 1.0074x over previous
"""MoE feed-forward (B=4,S=2048,D=1024,F=2048,E=8,top-2) on 8 trn2 NeuronCores.

Strategy (expert-parallel, per sharding hint):
 - Host computes the top-2 softmax routing (tiny: [T,1024]@[1024,8]) and
   dispatches tokens: core e receives the tokens routed to expert e,
   transposed to [D, C] (C = max token count over experts, zero padded).
 - Device (per core): h1 = W1^T x, s = silu(h1), h3 = W3^T x, g = s*h3,
   y^T = (W2^T g) * w  (w = per-token combine weight, broadcast across
   partitions), streamed over column blocks of <=512 tokens.
   All matmuls fp16 x fp16 -> fp32 PSUM; y emitted fp16, combined on host
   in fp32 via unweighted scatter-add.
"""

import numpy as np

import concourse.bass as bass
import concourse.tile as tile
from concourse import bacc, mybir
from concourse.bass_utils import run_bass_kernel_spmd

B, S, D, F, E, TOPK = 4, 2048, 1024, 2048, 8, 2
N_CORES = 8
KD = D // 128   # 8 contraction tiles for D
KF = F // 128   # 16 contraction tiles for F

_nc_cache = {}


def _build_nc(C):
    """Build the per-core Bass program for token capacity C."""
    f16 = mybir.dt.float16
    f32 = mybir.dt.float32
    blocks = [512] * (C // 512) + ([C % 512] if C % 512 else [])

    nc = bacc.Bacc(None, target_bir_lowering=False, enable_partition_id=False)
    xT = nc.dram_tensor("xT", [D, C], f16, kind="ExternalInput")
    wt = nc.dram_tensor("wt", [1, C], f32, kind="ExternalInput")
    W1 = nc.dram_tensor("W1", [D, F], f16, kind="ExternalInput")
    W3 = nc.dram_tensor("W3", [D, F], f16, kind="ExternalInput")
    W2 = nc.dram_tensor("W2", [F, D], f16, kind="ExternalInput")
    yT = nc.dram_tensor("yT", [D, C], f16, kind="ExternalOutput")

    # [D, nb] slab of xT/yT viewed as [128, KD, nb] (partition-major tiles)
    def slab(t, c0, nb):
        return t[:, c0:c0 + nb].rearrange("(k p) n -> p k n", p=128)

    with tile.TileContext(nc) as tc:
        with (
            tc.tile_pool(name="wpool", bufs=1) as wpool,
            tc.tile_pool(name="xpool", bufs=2) as xpool,
            tc.tile_pool(name="gpool", bufs=2) as gpool,
            tc.tile_pool(name="spool", bufs=1) as spool,
            tc.tile_pool(name="ypool", bufs=2) as ypool,
            tc.tile_pool(name="wbpool", bufs=2) as wbpool,
            tc.tile_pool(name="psA", bufs=4, space="PSUM") as psA,
            tc.tile_pool(name="psB", bufs=2, space="PSUM") as psB,
            tc.tile_pool(name="psY", bufs=2, space="PSUM") as psY,
        ):
            # All data DMAs share one HW queue in emission order. The h1 pass
            # needs only W1 + x block 0, so emit those first (interleaved so
            # the first f-group's k-tiles land earliest), then W3/W2.
            nb0 = blocks[0]
            w1sb = []
            for k in range(KD):
                t = wpool.tile([128, F], f16, tag=f"w1_{k}")
                w1sb.append(t)
            x0 = xpool.tile([128, KD, 512], f16, tag="x")
            nc.sync.dma_start(out=w1sb[0], in_=W1[0:128, :])
            nc.sync.dma_start(out=x0[:, 0:4, :nb0], in_=slab(xT, 0, nb0)[:, 0:4, :])
            for k in range(1, 4):
                nc.sync.dma_start(out=w1sb[k], in_=W1[k * 128:(k + 1) * 128, :])
            nc.sync.dma_start(out=x0[:, 4:KD, :nb0], in_=slab(xT, 0, nb0)[:, 4:KD, :])
            for k in range(4, KD):
                nc.sync.dma_start(out=w1sb[k], in_=W1[k * 128:(k + 1) * 128, :])

            wb0 = wbpool.tile([128, 512], f32, tag="wb")
            nc.sync.dma_start(
                out=wb0[:, :nb0],
                in_=bass.AP(tensor=wt.ap().tensor, offset=0,
                            ap=[[0, 128], [1, nb0]]),
            )

            w3sb = wpool.tile([128, KD, F], f16, tag="w3")
            nc.sync.dma_start(out=w3sb, in_=W3[:, :].rearrange("(k p) n -> p k n", p=128))
            w2sb = wpool.tile([128, KF, D], f16, tag="w2")
            nc.sync.dma_start(out=w2sb, in_=W2[:, :].rearrange("(k p) n -> p k n", p=128))

            c0 = 0
            for b, nb in enumerate(blocks):
                if b == 0:
                    xsb, wb = x0, wb0
                else:
                    xsb = xpool.tile([128, KD, 512], f16, tag="x")
                    nc.sync.dma_start(out=xsb[:, :, :nb], in_=slab(xT, c0, nb))
                    wb = wbpool.tile([128, 512], f32, tag="wb")
                    nc.sync.dma_start(
                        out=wb[:, :nb],
                        in_=bass.AP(tensor=wt.ap().tensor, offset=c0,
                                    ap=[[0, 128], [1, nb]]),
                    )

                # Pass 1: h1 = W1^T x, s = silu(h1)  (needs only W1 + x)
                sts = [None] * KF
                if b == 0:
                    # k-outer over the first 4 f-tiles: each W1 k-tile that
                    # lands from HBM immediately feeds 4 matmuls, absorbing
                    # the W1 streaming latency instead of stalling on f=0.
                    pss = [
                        psA.tile([128, 512], f32, tag="ps1", name=f"ps1w{f}")
                        for f in range(4)
                    ]
                    for k in range(KD):
                        for f in range(4):
                            fs = slice(f * 128, (f + 1) * 128)
                            nc.tensor.matmul(
                                pss[f][:, :nb], lhsT=w1sb[k][:, fs],
                                rhs=xsb[:, k, :nb],
                                start=(k == 0), stop=(k == KD - 1),
                            )
                    for f in range(4):
                        s = spool.tile([128, 512], f16, tag=f"s{f}")
                        nc.scalar.activation(
                            s[:, :nb], pss[f][:, :nb],
                            mybir.ActivationFunctionType.Silu,
                        )
                        sts[f] = s
                for f in range(4 if b == 0 else 0, KF):
                    fs = slice(f * 128, (f + 1) * 128)
                    ps1 = psA.tile([128, 512], f32, tag="ps1")
                    for k in range(KD):
                        nc.tensor.matmul(
                            ps1[:, :nb], lhsT=w1sb[k][:, fs], rhs=xsb[:, k, :nb],
                            start=(k == 0), stop=(k == KD - 1),
                        )
                    s = spool.tile([128, 512], f16, tag=f"s{f}")
                    nc.scalar.activation(
                        s[:, :nb], ps1[:, :nb], mybir.ActivationFunctionType.Silu
                    )
                    sts[f] = s

                # Pass 2: h3 = W3^T x, g = s * h3
                gts = []
                for f in range(KF):
                    fs = slice(f * 128, (f + 1) * 128)
                    ps3 = psB.tile([128, 512], f32, tag="ps3")
                    for k in range(KD):
                        nc.tensor.matmul(
                            ps3[:, :nb], lhsT=w3sb[:, k, fs], rhs=xsb[:, k, :nb],
                            start=(k == 0), stop=(k == KD - 1),
                        )
                    g = gpool.tile([128, 512], f16, tag=f"g{f}")
                    nc.vector.tensor_mul(g[:, :nb], sts[f][:, :nb], ps3[:, :nb])
                    gts.append(g)

                # Pass 3: y^T = (W2^T g) * w
                ysb = ypool.tile([128, KD, 512], f16, tag="y")
                for dd in range(KD):
                    ds_ = slice(dd * 128, (dd + 1) * 128)
                    psy = psY.tile([128, 512], f32, tag="psy")
                    for f in range(KF):
                        nc.tensor.matmul(
                            psy[:, :nb], lhsT=w2sb[:, f, ds_], rhs=gts[f][:, :nb],
                            start=(f == 0), stop=(f == KF - 1),
                        )
                    nc.vector.tensor_mul(ysb[:, dd, :nb], psy[:, :nb], wb[:, :nb])
                    if dd == KD // 2 - 1:
                        nc.sync.dma_start(
                            out=slab(yT, c0, nb)[:, 0:KD // 2, :],
                            in_=ysb[:, 0:KD // 2, :nb],
                        )
                nc.sync.dma_start(
                    out=slab(yT, c0, nb)[:, KD // 2:KD, :],
                    in_=ysb[:, KD // 2:KD, :nb],
                )
                c0 += nb
    nc.finalize()
    return nc


def _route(x, Wg):
    """Top-2 softmax routing in float64 (top-2/top-3 gaps are >>f32 eps, so
    this matches the f32 reference selection exactly)."""
    logits = x.astype(np.float64) @ Wg.astype(np.float64)
    logits -= logits.max(axis=-1, keepdims=True)
    g = np.exp(logits)
    g /= g.sum(axis=-1, keepdims=True)
    top_i = np.argpartition(-g, TOPK - 1, axis=-1)[:, :TOPK]      # [T, 2]
    tg = np.take_along_axis(g, top_i, axis=-1)
    tg = tg / tg.sum(axis=-1, keepdims=True)
    return top_i, tg


def run(inputs, trace=False, trace_cores=None):
    hidden_states = np.asarray(inputs["hidden_states"], dtype=np.float32)
    Wg = np.asarray(inputs["Wg"], dtype=np.float32)
    W1 = np.asarray(inputs["W1"], dtype=np.float32)
    W3 = np.asarray(inputs["W3"], dtype=np.float32)
    W2 = np.asarray(inputs["W2"], dtype=np.float32)

    x = hidden_states.reshape(-1, D)                              # [T, D]
    T = x.shape[0]
    top_i, tg = _route(x, Wg)

    idx = []
    wts = []
    for e in range(E):
        sel = top_i == e                                          # [T, 2]
        rows = np.where(sel.any(axis=-1))[0]
        idx.append(rows)
        wts.append(np.where(sel[rows, 0], tg[rows, 0], tg[rows, 1]))
    counts = [len(r) for r in idx]
    C = max(max(counts), 1)

    if C not in _nc_cache:
        _nc_cache[C] = _build_nc(C)
    nc = _nc_cache[C]

    in_maps = []
    for e in range(E):
        rows = idx[e]
        c = len(rows)
        xg = x[rows]                                              # [c, D] f32
        xTe = np.zeros((D, C), np.float16)
        xTe[:, :c] = xg.T
        wte = np.zeros((1, C), np.float32)
        wte[0, :c] = wts[e]
        in_maps.append({
            "xT": xTe,
            "wt": wte,
            "W1": W1[e].astype(np.float16),
            "W3": W3[e].astype(np.float16),
            "W2": W2[e].astype(np.float16),
        })

    kwargs = {}
    if trace:
        kwargs["trace"] = True
        kwargs["trace_cores"] = trace_cores or list(range(N_CORES))
    res = run_bass_kernel_spmd(nc, in_maps, list(range(N_CORES)), **kwargs)

    out = np.zeros((T, D), np.float32)
    for e in range(E):
        c = len(idx[e])
        if c:
            out[idx[e]] += res.results[e]["yT"][:, :c].T.astype(np.float32)
    return out.reshape(B, S, D), res


def kernel(**inputs):
    out, _ = run(inputs, trace=False)
    return out


# revision 13
# speedup vs baseline: 1.1511x; 1.0107x over previous
"""MoE feed-forward (B=4,S=2048,D=1024,F=2048,E=8,top-2) on 8 trn2 NeuronCores.

Strategy (expert-parallel, per sharding hint):
 - Host computes the top-2 softmax routing (tiny: [T,1024]@[1024,8]) and
   dispatches tokens: core e receives the tokens routed to expert e,
   transposed to [D, C] (C = max token count over experts, zero padded).
 - Device (per core): h1 = W1^T x, s = silu(h1), h3 = W3^T x, g = s*h3,
   y^T = (W2^T g) * w  (w = per-token combine weight, broadcast across
   partitions), streamed over column blocks of <=512 tokens.
   All matmuls fp16 x fp16 -> fp32 PSUM; y emitted fp16, combined on host
   in fp32 via unweighted scatter-add.
"""

import numpy as np

import concourse.bass as bass
import concourse.tile as tile
from concourse import bacc, mybir
from concourse.bass_utils import run_bass_kernel_spmd

B, S, D, F, E, TOPK = 4, 2048, 1024, 2048, 8, 2
N_CORES = 8
KD = D // 128   # 8 contraction tiles for D
KF = F // 128   # 16 contraction tiles for F

_nc_cache = {}


def _build_nc(C):
    """Build the per-core Bass program for token capacity C."""
    f16 = mybir.dt.float16
    f32 = mybir.dt.float32
    blocks = [512] * (C // 512) + ([C % 512] if C % 512 else [])

    nc = bacc.Bacc(None, target_bir_lowering=False, enable_partition_id=False)
    xT = nc.dram_tensor("xT", [D, C], f16, kind="ExternalInput")
    wt = nc.dram_tensor("wt", [1, C], f32, kind="ExternalInput")
    W1 = nc.dram_tensor("W1", [D, F], f16, kind="ExternalInput")
    W3 = nc.dram_tensor("W3", [D, F], f16, kind="ExternalInput")
    W2 = nc.dram_tensor("W2", [F, D], f16, kind="ExternalInput")
    yT = nc.dram_tensor("yT", [D, C], f16, kind="ExternalOutput")

    # [D, nb] slab of xT/yT viewed as [128, KD, nb] (partition-major tiles)
    def slab(t, c0, nb):
        return t[:, c0:c0 + nb].rearrange("(k p) n -> p k n", p=128)

    with tile.TileContext(nc) as tc:
        with (
            tc.tile_pool(name="wpool", bufs=1) as wpool,
            tc.tile_pool(name="xpool", bufs=2) as xpool,
            tc.tile_pool(name="gpool", bufs=2) as gpool,
            tc.tile_pool(name="spool", bufs=1) as spool,
            tc.tile_pool(name="ypool", bufs=2) as ypool,
            tc.tile_pool(name="wbpool", bufs=2) as wbpool,
            tc.tile_pool(name="psA", bufs=4, space="PSUM") as psA,
            tc.tile_pool(name="psB", bufs=2, space="PSUM") as psB,
            tc.tile_pool(name="psY", bufs=2, space="PSUM") as psY,
        ):
            # All data DMAs share one HW queue in emission order. The h1 pass
            # needs only W1 + x block 0, so emit those first (interleaved so
            # the first f-group's k-tiles land earliest), then W3/W2.
            nb0 = blocks[0]
            w1sb = []
            for k in range(KD):
                t = wpool.tile([128, F], f16, tag=f"w1_{k}")
                w1sb.append(t)
            x0 = xpool.tile([128, KD, 512], f16, tag="x")
            nc.sync.dma_start(out=w1sb[0], in_=W1[0:128, :])
            nc.sync.dma_start(out=x0[:, 0:4, :nb0], in_=slab(xT, 0, nb0)[:, 0:4, :])
            for k in range(1, 4):
                nc.sync.dma_start(out=w1sb[k], in_=W1[k * 128:(k + 1) * 128, :])
            nc.sync.dma_start(out=x0[:, 4:KD, :nb0], in_=slab(xT, 0, nb0)[:, 4:KD, :])
            for k in range(4, KD):
                nc.sync.dma_start(out=w1sb[k], in_=W1[k * 128:(k + 1) * 128, :])

            wb0 = wbpool.tile([128, 512], f32, tag="wb")
            nc.sync.dma_start(
                out=wb0[:, :nb0],
                in_=bass.AP(tensor=wt.ap().tensor, offset=0,
                            ap=[[0, 128], [1, nb0]]),
            )

            w3sb = wpool.tile([128, KD, F], f16, tag="w3")
            nc.sync.dma_start(out=w3sb, in_=W3[:, :].rearrange("(k p) n -> p k n", p=128))
            w2sb = wpool.tile([128, KF, D], f16, tag="w2")
            nc.sync.dma_start(out=w2sb, in_=W2[:, :].rearrange("(k p) n -> p k n", p=128))

            c0 = 0
            for b, nb in enumerate(blocks):
                if b == 0:
                    xsb, wb = x0, wb0
                else:
                    xsb = xpool.tile([128, KD, 512], f16, tag="x")
                    nc.sync.dma_start(out=xsb[:, :, :nb], in_=slab(xT, c0, nb))
                    wb = wbpool.tile([128, 512], f32, tag="wb")
                    nc.sync.dma_start(
                        out=wb[:, :nb],
                        in_=bass.AP(tensor=wt.ap().tensor, offset=c0,
                                    ap=[[0, 128], [1, nb]]),
                    )

                # Pass 1: h1 = W1^T x, s = silu(h1)  (needs only W1 + x)
                sts = [None] * KF
                if b == 0:
                    # k-outer over the first 8 f-tiles using all 8 PSUM banks
                    # (psB/psY tiles are free this early): each W1 k-tile that
                    # lands from HBM immediately feeds 8 matmuls, so the whole
                    # warmup runs under the W1 stream instead of stalling.
                    pss = [
                        psA.tile([128, 512], f32, tag="ps1", name=f"ps1w{f}")
                        for f in range(4)
                    ] + [
                        psB.tile([128, 512], f32, tag="ps3", name=f"ps3w{f}")
                        for f in range(2)
                    ] + [
                        psY.tile([128, 512], f32, tag="psy", name=f"psyw{f}")
                        for f in range(2)
                    ]
                    for k in range(KD):
                        for f in range(8):
                            fs = slice(f * 128, (f + 1) * 128)
                            nc.tensor.matmul(
                                pss[f][:, :nb], lhsT=w1sb[k][:, fs],
                                rhs=xsb[:, k, :nb],
                                start=(k == 0), stop=(k == KD - 1),
                            )
                    for f in range(8):
                        s = spool.tile([128, 512], f16, tag=f"s{f}")
                        nc.scalar.activation(
                            s[:, :nb], pss[f][:, :nb],
                            mybir.ActivationFunctionType.Silu,
                        )
                        sts[f] = s
                for f in range(8 if b == 0 else 0, KF):
                    fs = slice(f * 128, (f + 1) * 128)
                    ps1 = psA.tile([128, 512], f32, tag="ps1")
                    for k in range(KD):
                        nc.tensor.matmul(
                            ps1[:, :nb], lhsT=w1sb[k][:, fs], rhs=xsb[:, k, :nb],
                            start=(k == 0), stop=(k == KD - 1),
                        )
                    s = spool.tile([128, 512], f16, tag=f"s{f}")
                    nc.scalar.activation(
                        s[:, :nb], ps1[:, :nb], mybir.ActivationFunctionType.Silu
                    )
                    sts[f] = s

                # Pass 2: h3 = W3^T x, g = s * h3
                gts = []
                for f in range(KF):
                    fs = slice(f * 128, (f + 1) * 128)
                    ps3 = psB.tile([128, 512], f32, tag="ps3")
                    for k in range(KD):
                        nc.tensor.matmul(
                            ps3[:, :nb], lhsT=w3sb[:, k, fs], rhs=xsb[:, k, :nb],
                            start=(k == 0), stop=(k == KD - 1),
                        )
                    g = gpool.tile([128, 512], f16, tag=f"g{f}")
                    nc.vector.tensor_mul(g[:, :nb], sts[f][:, :nb], ps3[:, :nb])
                    gts.append(g)

                # Pass 3: y^T = (W2^T g) * w
                ysb = ypool.tile([128, KD, 512], f16, tag="y")
                for dd in range(KD):
                    ds_ = slice(dd * 128, (dd + 1) * 128)
                    psy = psY.tile([128, 512], f32, tag="psy")
                    for f in range(KF):
                        nc.tensor.matmul(
                            psy[:, :nb], lhsT=w2sb[:, f, ds_], rhs=gts[f][:, :nb],
                            start=(f == 0), stop=(f == KF - 1),
                        )
                    nc.vector.tensor_mul(ysb[:, dd, :nb], psy[:, :nb], wb[:, :nb])
                    if dd == KD // 2 - 1:
                        nc.sync.dma_start(
                            out=slab(yT, c0, nb)[:, 0:KD // 2, :],
                            in_=ysb[:, 0:KD // 2, :nb],
                        )
                nc.sync.dma_start(
                    out=slab(yT, c0, nb)[:, KD // 2:KD, :],
                    in_=ysb[:, KD // 2:KD, :nb],
                )
                c0 += nb
    nc.finalize()
    return nc


def _route(x, Wg):
    """Top-2 softmax routing in float64 (top-2/top-3 gaps are >>f32 eps, so
    this matches the f32 reference selection exactly)."""
    logits = x.astype(np.float64) @ Wg.astype(np.float64)
    logits -= logits.max(axis=-1, keepdims=True)
    g = np.exp(logits)
    g /= g.sum(axis=-1, keepdims=True)
    top_i = np.argpartition(-g, TOPK - 1, axis=-1)[:, :TOPK]      # [T, 2]
    tg = np.take_along_axis(g, top_i, axis=-1)
    tg = tg / tg.sum(axis=-1, keepdims=True)
    return top_i, tg


def run(inputs, trace=False, trace_cores=None):
    hidden_states = np.asarray(inputs["hidden_states"], dtype=np.float32)
    Wg = np.asarray(inputs["Wg"], dtype=np.float32)
    W1 = np.asarray(inputs["W1"], dtype=np.float32)
    W3 = np.asarray(inputs["W3"], dtype=np.float32)
    W2 = np.asarray(inputs["W2"], dtype=np.float32)

    x = hidden_states.reshape(-1, D)                              # [T, D]
    T = x.shape[0]
    top_i, tg = _route(x, Wg)

    idx = []
    wts = []
    for e in range(E):
        sel = top_i == e                                          # [T, 2]
        rows = np.where(sel.any(axis=-1))[0]
        idx.append(rows)
        wts.append(np.where(sel[rows, 0], tg[rows, 0], tg[rows, 1]))
    counts = [len(r) for r in idx]
    C = max(max(counts), 1)

    if C not in _nc_cache:
        _nc_cache[C] = _build_nc(C)
    nc = _nc_cache[C]

    in_maps = []
    for e in range(E):
        rows = idx[e]
        c = len(rows)
        xg = x[rows]                                              # [c, D] f32
        xTe = np.zeros((D, C), np.float16)
        xTe[:, :c] = xg.T
        wte = np.zeros((1, C), np.float32)
        wte[0, :c] = wts[e]
        in_maps.append({
            "xT": xTe,
            "wt": wte,
            "W1": W1[e].astype(np.float16),
            "W3": W3[e].astype(np.float16),
            "W2": W2[e].astype(np.float16),
        })

    kwargs = {}
    if trace:
        kwargs["trace"] = True
        kwargs["trace_cores"] = trace_cores or list(range(N_CORES))
    res = run_bass_kernel_spmd(nc, in_maps, list(range(N_CORES)), **kwargs)

    out = np.zeros((T, D), np.float32)
    for e in range(E):
        c = len(idx[e])
        if c:
            out[idx[e]] += res.results[e]["yT"][:, :c].T.astype(np.float32)
    return out.reshape(B, S, D), res


def kernel(**inputs):
    out, _ = run(inputs, trace=False)
    return out


# revision 14
# speedup vs baseline: 1.1541x; 1.0026x over previous
"""MoE feed-forward (B=4,S=2048,D=1024,F=2048,E=8,top-2) on 8 trn2 NeuronCores.

Strategy (expert-parallel, per sharding hint):
 - Host computes the top-2 softmax routing (tiny: [T,1024]@[1024,8]) and
   dispatches tokens: core e receives the tokens routed to expert e,
   transposed to [D, C] (C = max token count over experts, zero padded).
 - Device (per core): h1 = W1^T x, s = silu(h1), h3 = W3^T x, g = s*h3,
   y^T = (W2^T g) * w  (w = per-token combine weight, broadcast across
   partitions), streamed over column blocks of <=512 tokens.
   All matmuls fp16 x fp16 -> fp32 PSUM; y emitted fp16, combined on host
   in fp32 via unweighted scatter-add.
"""

import numpy as np

import concourse.bass as bass
import concourse.tile as tile
from concourse import bacc, mybir
from concourse.bass_utils import run_bass_kernel_spmd

B, S, D, F, E, TOPK = 4, 2048, 1024, 2048, 8, 2
N_CORES = 8
KD = D // 128   # 8 contraction tiles for D
KF = F // 128   # 16 contraction tiles for F

_nc_cache = {}


def _build_nc(C):
    """Build the per-core Bass program for token capacity C."""
    f16 = mybir.dt.float16
    f32 = mybir.dt.float32
    blocks = [512] * (C // 512) + ([C % 512] if C % 512 else [])

    nc = bacc.Bacc(None, target_bir_lowering=False, enable_partition_id=False)
    xT = nc.dram_tensor("xT", [D, C], f16, kind="ExternalInput")
    wt = nc.dram_tensor("wt", [1, C], f32, kind="ExternalInput")
    W1 = nc.dram_tensor("W1", [D, F], f16, kind="ExternalInput")
    W3 = nc.dram_tensor("W3", [D, F], f16, kind="ExternalInput")
    W2 = nc.dram_tensor("W2", [F, D], f16, kind="ExternalInput")
    yT = nc.dram_tensor("yT", [D, C], f16, kind="ExternalOutput")

    # [D, nb] slab of xT/yT viewed as [128, KD, nb] (partition-major tiles)
    def slab(t, c0, nb):
        return t[:, c0:c0 + nb].rearrange("(k p) n -> p k n", p=128)

    with tile.TileContext(nc) as tc:
        with (
            tc.tile_pool(name="wpool", bufs=1) as wpool,
            tc.tile_pool(name="xpool", bufs=2) as xpool,
            tc.tile_pool(name="gpool", bufs=2) as gpool,
            tc.tile_pool(name="spool", bufs=1) as spool,
            tc.tile_pool(name="ypool", bufs=2) as ypool,
            tc.tile_pool(name="wbpool", bufs=2) as wbpool,
            tc.tile_pool(name="psA", bufs=4, space="PSUM") as psA,
            tc.tile_pool(name="psB", bufs=2, space="PSUM") as psB,
            tc.tile_pool(name="psY", bufs=2, space="PSUM") as psY,
        ):
            # All data DMAs share one HW queue in emission order. The h1 pass
            # needs only W1 + x block 0, so emit those first (interleaved so
            # the first f-group's k-tiles land earliest), then W3/W2.
            nb0 = blocks[0]
            w1sb = []
            for k in range(KD):
                t = wpool.tile([128, F], f16, tag=f"w1_{k}")
                w1sb.append(t)
            x0 = xpool.tile([128, KD, 512], f16, tag="x")
            nc.sync.dma_start(out=w1sb[0], in_=W1[0:128, :])
            nc.sync.dma_start(out=x0[:, 0:4, :nb0], in_=slab(xT, 0, nb0)[:, 0:4, :])
            for k in range(1, 4):
                nc.sync.dma_start(out=w1sb[k], in_=W1[k * 128:(k + 1) * 128, :])
            nc.sync.dma_start(out=x0[:, 4:KD, :nb0], in_=slab(xT, 0, nb0)[:, 4:KD, :])
            for k in range(4, KD):
                nc.sync.dma_start(out=w1sb[k], in_=W1[k * 128:(k + 1) * 128, :])

            wb0 = wbpool.tile([128, 512], f32, tag="wb")
            nc.sync.dma_start(
                out=wb0[:, :nb0],
                in_=bass.AP(tensor=wt.ap().tensor, offset=0,
                            ap=[[0, 128], [1, nb0]]),
            )

            w3sb = wpool.tile([128, KD, F], f16, tag="w3")
            nc.sync.dma_start(out=w3sb, in_=W3[:, :].rearrange("(k p) n -> p k n", p=128))
            w2sb = wpool.tile([128, KF, D], f16, tag="w2")
            nc.sync.dma_start(out=w2sb, in_=W2[:, :].rearrange("(k p) n -> p k n", p=128))

            c0 = 0
            for b, nb in enumerate(blocks):
                if b == 0:
                    xsb, wb = x0, wb0
                else:
                    xsb = xpool.tile([128, KD, 512], f16, tag="x")
                    nc.sync.dma_start(out=xsb[:, :, :nb], in_=slab(xT, c0, nb))
                    wb = wbpool.tile([128, 512], f32, tag="wb")
                    nc.sync.dma_start(
                        out=wb[:, :nb],
                        in_=bass.AP(tensor=wt.ap().tensor, offset=c0,
                                    ap=[[0, 128], [1, nb]]),
                    )

                # Pass 1: h1 = W1^T x, s = silu(h1)  (needs only W1 + x)
                sts = [None] * KF
                if b == 0:
                    # k-outer over the first 8 f-tiles using all 8 PSUM banks
                    # (psB/psY tiles are free this early): each W1 k-tile that
                    # lands from HBM immediately feeds 8 matmuls, so the whole
                    # warmup runs under the W1 stream instead of stalling.
                    pss = [
                        psA.tile([128, 512], f32, tag="ps1", name=f"ps1w{f}")
                        for f in range(4)
                    ] + [
                        psB.tile([128, 512], f32, tag="ps3", name=f"ps3w{f}")
                        for f in range(2)
                    ] + [
                        psY.tile([128, 512], f32, tag="psy", name=f"psyw{f}")
                        for f in range(2)
                    ]
                    for k in range(KD):
                        for f in range(8):
                            fs = slice(f * 128, (f + 1) * 128)
                            nc.tensor.matmul(
                                pss[f][:, :nb], lhsT=w1sb[k][:, fs],
                                rhs=xsb[:, k, :nb],
                                start=(k == 0), stop=(k == KD - 1),
                            )
                    for f in range(8):
                        s = spool.tile([128, 512], f16, tag=f"s{f}")
                        nc.scalar.activation(
                            s[:, :nb], pss[f][:, :nb],
                            mybir.ActivationFunctionType.Silu,
                        )
                        sts[f] = s
                for f in range(8 if b == 0 else 0, KF):
                    fs = slice(f * 128, (f + 1) * 128)
                    ps1 = psA.tile([128, 512], f32, tag="ps1")
                    for k in range(KD):
                        nc.tensor.matmul(
                            ps1[:, :nb], lhsT=w1sb[k][:, fs], rhs=xsb[:, k, :nb],
                            start=(k == 0), stop=(k == KD - 1),
                        )
                    s = spool.tile([128, 512], f16, tag=f"s{f}")
                    nc.scalar.activation(
                        s[:, :nb], ps1[:, :nb], mybir.ActivationFunctionType.Silu
                    )
                    sts[f] = s

                # Pass 2: h3 = W3^T x, g = s * h3
                gts = []
                for f in range(KF):
                    fs = slice(f * 128, (f + 1) * 128)
                    ps3 = psB.tile([128, 512], f32, tag="ps3")
                    for k in range(KD):
                        nc.tensor.matmul(
                            ps3[:, :nb], lhsT=w3sb[:, k, fs], rhs=xsb[:, k, :nb],
                            start=(k == 0), stop=(k == KD - 1),
                        )
                    g = gpool.tile([128, 512], f16, tag=f"g{f}")
                    nc.vector.tensor_mul(g[:, :nb], sts[f][:, :nb], ps3[:, :nb])
                    gts.append(g)

                # Pass 3: y^T = (W2^T g) * w
                ysb = ypool.tile([128, KD, 512], f16, tag="y")
                for dd in range(KD):
                    ds_ = slice(dd * 128, (dd + 1) * 128)
                    psy = psY.tile([128, 512], f32, tag="psy")
                    for f in range(KF):
                        nc.tensor.matmul(
                            psy[:, :nb], lhsT=w2sb[:, f, ds_], rhs=gts[f][:, :nb],
                            start=(f == 0), stop=(f == KF - 1),
                        )
                    nc.vector.tensor_mul(ysb[:, dd, :nb], psy[:, :nb], wb[:, :nb])
                    if b == len(blocks) - 1:
                        # last block: per-tile output DMAs so the kernel-tail
                        # drain only waits on a tiny final transfer
                        nc.sync.dma_start(
                            out=slab(yT, c0, nb)[:, dd:dd + 1, :],
                            in_=ysb[:, dd:dd + 1, :nb],
                        )
                    elif dd == KD // 2 - 1:
                        nc.sync.dma_start(
                            out=slab(yT, c0, nb)[:, 0:KD // 2, :],
                            in_=ysb[:, 0:KD // 2, :nb],
                        )
                if b != len(blocks) - 1:
                    nc.sync.dma_start(
                        out=slab(yT, c0, nb)[:, KD // 2:KD, :],
                        in_=ysb[:, KD // 2:KD, :nb],
                    )
                c0 += nb
    nc.finalize()
    return nc


def _route(x, Wg):
    """Top-2 softmax routing in float64 (top-2/top-3 gaps are >>f32 eps, so
    this matches the f32 reference selection exactly)."""
    logits = x.astype(np.float64) @ Wg.astype(np.float64)
    logits -= logits.max(axis=-1, keepdims=True)
    g = np.exp(logits)
    g /= g.sum(axis=-1, keepdims=True)
    top_i = np.argpartition(-g, TOPK - 1, axis=-1)[:, :TOPK]      # [T, 2]
    tg = np.take_along_axis(g, top_i, axis=-1)
    tg = tg / tg.sum(axis=-1, keepdims=True)
    return top_i, tg


def run(inputs, trace=False, trace_cores=None):
    hidden_states = np.asarray(inputs["hidden_states"], dtype=np.float32)
    Wg = np.asarray(inputs["Wg"], dtype=np.float32)
    W1 = np.asarray(inputs["W1"], dtype=np.float32)
    W3 = np.asarray(inputs["W3"], dtype=np.float32)
    W2 = np.asarray(inputs["W2"], dtype=np.float32)

    x = hidden_states.reshape(-1, D)                              # [T, D]
    T = x.shape[0]
    top_i, tg = _route(x, Wg)

    idx = []
    wts = []
    for e in range(E):
        sel = top_i == e                                          # [T, 2]
        rows = np.where(sel.any(axis=-1))[0]
        idx.append(rows)
        wts.append(np.where(sel[rows, 0], tg[rows, 0], tg[rows, 1]))
    counts = [len(r) for r in idx]
    C = max(max(counts), 1)

    if C not in _nc_cache:
        _nc_cache[C] = _build_nc(C)
    nc = _nc_cache[C]

    in_maps = []
    for e in range(E):
        rows = idx[e]
        c = len(rows)
        xg = x[rows]                                              # [c, D] f32
        xTe = np.zeros((D, C), np.float16)
        xTe[:, :c] = xg.T
        wte = np.zeros((1, C), np.float32)
        wte[0, :c] = wts[e]
        in_maps.append({
            "xT": xTe,
            "wt": wte,
            "W1": W1[e].astype(np.float16),
            "W3": W3[e].astype(np.float16),
            "W2": W2[e].astype(np.float16),
        })

    kwargs = {}
    if trace:
        kwargs["trace"] = True
        kwargs["trace_cores"] = trace_cores or list(range(N_CORES))
    res = run_bass_kernel_spmd(nc, in_maps, list(range(N_CORES)), **kwargs)

    out = np.zeros((T, D), np.float32)
    for e in range(E):
        c = len(idx[e])
        if c:
            out[idx[e]] += res.results[e]["yT"][:, :c].T.astype(np.float32)
    return out.reshape(B, S, D), res


def kernel(**inputs):
    out, _ = run(inputs, trace=False)
    return out
